# revision 1
# baseline (speedup 1.0000x reference)
"""PeakLocalMax (41x41 NMS mask) Trainium2 Bass kernel.

Input : batch_heatmap (16, 1024, 1024, 2) float32
Output: bool mask, same shape: (x == maxpool41x41(x)) & (x > 0.5)

Strategy (per core; batch sharded 2 images/core over 8 cores):
  - Exact f32 separable sliding-window max via van Herk/Gil-Werman:
    segmented prefix/suffix max scans implemented with tensor_tensor_scan
    (op0=min with a block-reset mask, op1=max) + a fused 3-way max
    (scalar_tensor_tensor) combine that also folds in the 0.5 threshold
    as c = nextafter(0.5): M2 = max(window_max, c); out = (x >= M2).
    Un-padded block grid; edge-window clipping comes from persistent
    constant-c margins on the scan-output tiles, so each 1D pass is
    exactly 3 DVE instructions (fwd scan, reversed scan, STT combine).
  - W-direction pass on (H=partitions, W*C=free) tiles using stride-2
    channel views; H-direction pass on PE-transposed strips
    (W=partitions, H=free); transpose back via PE and compare reading
    PSUM directly.  All scan/STT work is VectorE-only: walrus rejects
    these opcodes (and even plain tensor_tensor) on GpSimd in ISA v3.
"""

import os
import sys
import numpy as np

_TRN_REPO = "/opt/trn_rl_repo"

H = 1024
W = 1024
C = 2
B_PER_CORE = 2
N_CORES = 8
V = 20            # min_distance
WIN = 2 * V + 1   # 41
HB = H // 128     # 8 h-blocks
WB = W // 128     # 8 w-blocks
BIG = float(np.float32(3e38))
C05 = float(np.nextafter(np.float32(0.5), np.float32(1)))

_CACHE = {}


def _build():
    if _TRN_REPO not in sys.path:
        sys.path.insert(0, _TRN_REPO)
    from contextlib import ExitStack
    from concourse import bacc, mybir
    import concourse.tile as tile
    from concourse.masks import make_identity
    from concourse.bass import _add_dep_helper

    F32 = mybir.dt.float32
    U8 = mybir.dt.uint8
    Alu = mybir.AluOpType

    nc = bacc.Bacc("TRN2", debug=False, num_devices=N_CORES)
    x_d = nc.dram_tensor("x", [B_PER_CORE, H, W * C], F32, kind="ExternalInput").ap()
    y_d = nc.dram_tensor("y", [B_PER_CORE, H, W * C], U8, kind="ExternalOutput").ap()

    # alternate scan/combine engine assignment DVE/GPSIMD 1:1
    cnt = [0]

    with tile.TileContext(nc) as tc, ExitStack() as ctx:
        sb = ctx.enter_context(tc.tile_pool(name="sb", bufs=1))
        xpool = ctx.enter_context(tc.tile_pool(name="xp", bufs=1))
        spool = ctx.enter_context(tc.tile_pool(name="sp", bufs=1))
        ps = ctx.enter_context(tc.tile_pool(name="ps", bufs=1, space="PSUM"))

        def eng():
            # scans/STT are DVE-only ops in ISA v3 (walrus rejects them on
            # the POOL engine); GPSIMD offload must use plain tensor ops.
            cnt[0] += 1
            return nc.vector

        # constants: scan reset masks + PE identity
        # fwd: reset at k % 41 == 0 ; rev (suffix, scanned backwards):
        # reset at k % 41 == 40 plus the truncated tail element 1023.
        mf = sb.tile([128, W], F32, name="mf")
        mr = sb.tile([128, W], F32, name="mr")
        idn = sb.tile([128, 128], F32, name="idn")
        nc.vector.memset(mf[:], BIG)
        nc.vector.memset(mf[:, 0:W:WIN], -BIG)
        nc.vector.memset(mr[:], BIG)
        nc.vector.memset(mr[:, V * 2:W:WIN], -BIG)
        nc.vector.memset(mr[:, W - 1:W], -BIG)
        make_identity(nc, idn[:])

        # Persistent scan-output tiles with constant-c margins so each
        # combine is a single full-width STT (no edge-clip ops):
        #   Se_ext = [ c*20 | suffix-scan(1024) ]  -> Se_ext[w] = S[w-20] or c
        #   Pe_ext = [ prefix-scan(1024) | c*20 ]  -> Pe_ext[w+20] = P[w+20] or c
        # Two of each (manual double-buffer via unit-counter parity).
        EXT = W + V
        scan_bufs = {}
        for nm in ("Pw", "Sw", "Ph", "Sh"):
            pair = []
            for i in range(2):
                t = sb.tile([128, EXT], F32, name=f"{nm}{i}")
                if nm[0] == "P":
                    nc.vector.memset(t[:, W:EXT], C05)
                else:
                    nc.vector.memset(t[:, 0:V], C05)
                pair.append(t)
            scan_bufs[nm] = pair

        # segmented scans + combine: out[w] = max(S[w-20], P[w+20], c),
        # window clipping supplied by the constant margins.
        def vh_pass(e, out_ap, data_ap, u, axis):
            Pe = scan_bufs["Pw" if axis == "w" else "Ph"][u % 2]
            Se = scan_bufs["Sw" if axis == "w" else "Sh"][u % 2]
            e.tensor_tensor_scan(Pe[:, 0:W], mf[:], data_ap,
                                 -BIG, op0=Alu.min, op1=Alu.max)
            e.tensor_tensor_scan(Se[:, V:EXT][:, ::-1], mr[:, ::-1],
                                 data_ap[:, ::-1],
                                 -BIG, op0=Alu.min, op1=Alu.max)
            e.scalar_tensor_tensor(out_ap, Se[:, 0:W], C05, Pe[:, V:EXT],
                                   op0=Alu.max, op1=Alu.max)

        # last reader instruction of each strip slot (for cross-image ordering)
        strip_last = {}

        for img in range(B_PER_CORE):
            strips = {}
            for ch in range(C):
                for wb in range(WB):
                    strips[(ch, wb)] = spool.tile(
                        [128, W], F32, name=f"st{img}_{ch}_{wb}",
                        tag=f"st{wb}", bufs=2)

            # ---- W-direction pass ----
            for hb in range(HB):
                xt = xpool.tile([128, W * C], F32, name=f"xt{img}_{hb}",
                                tag="xt", bufs=3)
                nc.sync.dma_start(xt[:], x_d[img, hb * 128:(hb + 1) * 128])
                for ch in range(C):
                    xv = xt[:, ch:W * C:2]         # (128, W) channel view
                    R = sb.tile([128, W], F32, name="R", tag="R", bufs=2)
                    e = eng()
                    vh_pass(e, R[:], xv, cnt[0], "w")
                    # transpose R into strips
                    for wb in range(WB):
                        pt = ps.tile([128, 128], F32, name="pt", tag="pt", bufs=3)
                        nc.tensor.transpose(pt[:], R[:, wb * 128:(wb + 1) * 128],
                                            idn[:])
                        cp = nc.scalar.copy(
                            strips[(ch, wb)][:, hb * 128:(hb + 1) * 128], pt[:])
                        if hb == 0:
                            key = (img - 1, ch, wb)
                            if key in strip_last:
                                _add_dep_helper(cp.ins, strip_last[key].ins,
                                                True, "strip slot reuse")

            # ---- H-direction pass (on transposed strips) ----
            for ch in range(C):
                for wb in range(WB):
                    st = strips[(ch, wb)]
                    e = eng()
                    # M2_T written in-place over the strip
                    vh_pass(e, st[:, 0:W], st[:], cnt[0], "h")

            # ---- transpose back + final compare + store ----
            for hb in range(HB):
                xc = sb.tile([128, W * C], F32, name="xc", tag="xc", bufs=3)
                nc.sync.dma_start(xc[:], x_d[img, hb * 128:(hb + 1) * 128])
                ot = sb.tile([128, W * C], U8, name="ot", tag="ot", bufs=3)
                for ch in range(C):
                    m2p = ps.tile([128, W], F32, name="m2p", tag="m2p", bufs=2)
                    for wb in range(WB):
                        tb = nc.tensor.transpose(
                            m2p[:, wb * 128:(wb + 1) * 128],
                            strips[(ch, wb)][:, hb * 128:(hb + 1) * 128],
                            idn[:])
                        if hb == HB - 1:
                            strip_last[(img, ch, wb)] = tb
                    nc.vector.tensor_tensor(
                        ot[:, ch:W * C:2],
                        xc[:, ch:W * C:2],
                        m2p[:], op=Alu.is_ge)
                nc.sync.dma_start(y_d[img, hb * 128:(hb + 1) * 128], ot[:])

    nc.compile()
    return nc


def _get_nc():
    if "nc" not in _CACHE:
        _CACHE["nc"] = _build()
    return _CACHE["nc"]


def _install_neff_cache():
    """Cache compiled NEFFs on disk keyed by BIR hash (compile is ~10 min)."""
    if _CACHE.get("neff_cache"):
        return
    import hashlib
    import shutil
    from concourse import bass_utils, bass2jax

    real = bass_utils.compile_bir_kernel
    cache_dir = "/tmp/bass_neff_cache"

    def cached(bir_json, tmpdir, neff_name="file.neff"):
        os.makedirs(cache_dir, exist_ok=True)
        key = hashlib.sha256(bir_json).hexdigest()[:32]
        hit = os.path.join(cache_dir, key + ".neff")
        dst = os.path.join(tmpdir, neff_name)
        if os.path.exists(hit):
            shutil.copyfile(hit, dst)
            return dst
        out = real(bir_json, tmpdir, neff_name)
        try:
            shutil.copyfile(out, hit)
        except OSError:
            pass
        return out

    bass_utils.compile_bir_kernel = cached
    if getattr(bass2jax, "compile_bir_kernel", None) is not None:
        bass2jax.compile_bir_kernel = cached
    _CACHE["neff_cache"] = True


def kernel(batch_heatmap: np.ndarray) -> np.ndarray:
    if _TRN_REPO not in sys.path:
        sys.path.insert(0, _TRN_REPO)
    from concourse.bass_utils import run_bass_kernel_spmd
    _install_neff_cache()

    x = np.ascontiguousarray(np.asarray(batch_heatmap, dtype=np.float32))
    assert x.shape == (16, H, W, C), x.shape
    nc = _get_nc()
    in_maps = [
        {"x": x[B_PER_CORE * r:B_PER_CORE * (r + 1)].reshape(B_PER_CORE, H, W * C)}
        for r in range(N_CORES)
    ]
    res = run_bass_kernel_spmd(nc, in_maps, list(range(N_CORES)))
    out = np.stack([res.results[r]["y"] for r in range(N_CORES)])
    return out.reshape(16, H, W, C).astype(bool)

